# revision 29
# baseline (speedup 1.0000x reference)
"""Multi-head causal attention (B=4, S=2048, D=2048, H=16) on 8 trn2 cores.

Sharding: core c handles batch b = c//2 and head-group g = c%2 (8 heads).
Each core computes q/k/v projections for its heads, causal attention, and a
partial out_proj over its dv-slice. Host sums the two partials per batch.

All matmuls in bfloat16 (same 1 cyc/row as f32r but half the bytes),
everything SBUF-resident (no DRAM round trip for q/k/v), input DMAs ordered
so the first projection chains consume x chunks as they stream in (single
in-order SP HWDGE queue), 256-wide query blocks with the upper diagonal
key chunk processed 128 wide (it is dead for the block's first 128
queries), out_proj emitted as per-head filler chains that cover softmax-exp
latency, and dep-free warm-up matmuls at t=0 that flip the PE HAM
clock-gate to 8/8 during the DMA queue spin-up window.

Device pipeline (per core):
  phase 1b: v[s, ev] = xT-chunks.T @ WvT, scaled by exp(alibi_bias[h, k])
            during PSUM evacuation (folds ALiBi into softmax via
            exp(s + b) = exp(s) * exp(b)). First 8 chains are emitted
            dc-outer so they consume x chunks as the DMA stream lands.
  phase 1a: qkT[e, s] = WqkT-chunks.T @ xT   (e: 8 q-heads then 8 k-heads)
  phase 2 per (256-query block, head): scoresT[k, q] = kT-chunk.T @ qT,
     four 128-key chunks into one [128, 1024] PSUM tile
     -> one wide ACT exp -> GPSIMD affine_select zeroes the causal
        staircase on the two diagonal chunks
     -> sumexp[*, q] += ebias-col-broadcast.T @ expT
     -> attnT[dv, q] += v'-chunk.T @ expT
     -> attnT *= 1/sumexp  (DVE reciprocal + mul)
  phase 3 (interleaved, one query block behind): O[s, e] partial
     = attnT-chunks.T @ out_projT over this core's dv-slice.
"""
import os
import sys
import types
from collections import deque

if "/opt/trn_rl_repo" not in sys.path:
    sys.path.insert(0, "/opt/trn_rl_repo")

import numpy as np

B, S, D, H = 4, 2048, 2048, 16
HD = D // H          # 128 head dim
HPC = H // 2         # 8 heads per core
EV = HPC * HD        # 1024 dv-slice per core
NKC = S // 128       # 16 key chunks
NDC = D // 128       # 16 contraction chunks
QW = 256             # query block width
NQB = S // QW        # 8 query blocks

_NC_CACHE = {}
LAST_EXEC_NS = None
LAST_PER_CORE_NS = None


def _install_ntff_hook():
    try:
        import antenv
        if "antenv.axon_hooks" in sys.modules:
            return
        mod = types.ModuleType("antenv.axon_hooks")
        state = {"hook": None}
        mod.set_axon_ntff_profile_hook = lambda h: state.__setitem__("hook", h)
        mod.get_axon_ntff_profile_hook = lambda: state["hook"]
        sys.modules["antenv.axon_hooks"] = mod
        antenv.axon_hooks = mod
        from trn_agent_boot.trn_boot import _ntff_profile_via_ctypes
        mod.set_axon_ntff_profile_hook(
            _ntff_profile_via_ctypes("/opt/axon/libaxon_pjrt.so"))
    except Exception:
        pass


def _build_nc():
    import concourse.bacc as bacc
    import concourse.mybir as mybir
    import concourse.tile as tile

    F32 = mybir.dt.float32
    BF16 = mybir.dt.bfloat16
    EXP = mybir.ActivationFunctionType.Exp
    MULT = mybir.AluOpType.mult
    GE = mybir.AluOpType.is_ge

    nc = bacc.Bacc()
    # xt[p, dc, s] = x[b, s, 128*dc+p]
    xt = nc.dram_tensor("xt", [128, NDC, S], BF16, kind="ExternalInput")
    # wqk[p, ec, dc, e] = Wqk_scaled[128*ec+e, 128*dc+p]
    wqk = nc.dram_tensor("wqk", [128, 16, NDC, 128], BF16,
                         kind="ExternalInput")
    # wv[p, evc, dc, c] = Wv[512*evc+c, 128*dc+p]
    wv = nc.dram_tensor("wv", [128, 2, NDC, 512], BF16, kind="ExternalInput")
    # ptt[p, dvc, e] = out_proj_w[e, 128*dvc+p]  (within this core's slice)
    ptt = nc.dram_tensor("ptt", [128, HPC, D], BF16, kind="ExternalInput")
    # ebias[i, h*16+kc] = exp(attn_bias[h, kc*128+i])
    ebias_r = nc.dram_tensor("ebias_r", [128, HPC * NKC], BF16,
                             kind="ExternalInput")
    ebias_f = nc.dram_tensor("ebias_f", [128, HPC * NKC], F32,
                             kind="ExternalInput")
    out = nc.dram_tensor("o", [S, D], BF16, kind="ExternalOutput")

    with tile.TileContext(nc) as tc:
        with (
            tc.tile_pool(name="consts", bufs=1) as cp,
            tc.tile_pool(name="qk", bufs=1) as qkp,
            tc.tile_pool(name="vv", bufs=1) as vp,
        ):
            ebr_t = cp.tile([128, HPC * NKC], BF16, tag="ebr", name="ebr")
            ebf_t = cp.tile([128, HPC * NKC], F32, tag="ebf", name="ebf")
            nc.sync.dma_start(ebr_t[:], ebias_r[:])
            nc.sync.dma_start(ebf_t[:], ebias_f[:])

            v_tiles = [vp.tile([128, EV], BF16, tag=f"v{sc}", name=f"v{sc}")
                       for sc in range(NKC)]
            qk_tiles = [qkp.tile([128, S], BF16, tag=f"qk{ec}",
                                 name=f"qk{ec}")
                        for ec in range(16)]

            # ---------------- phase 1: projections ----------------
            with (
                tc.tile_pool(name="xp", bufs=1) as xp,
                tc.tile_pool(name="wvp", bufs=1) as wvp,
                tc.tile_pool(name="wp", bufs=3) as wp,
                tc.tile_pool(name="ps1", bufs=8, space="PSUM") as pp,
            ):
                # warm-up: dep-free dummy matmuls fill the DMA queue spin-up
                # window and flip the PE HAM clock-gate to 8/8 before real
                # work arrives. Inputs are uninitialized SBUF; output unused.
                wu_t = xp.tile([128, 512], BF16, tag="wu", name="wu")
                nc.vector.memset(wu_t[:], 0.0)
                wu_ps = pp.tile([128, 512], F32, tag="p", name="wu_ps")
                for r in range(38):
                    nc.tensor.matmul(wu_ps[:], wu_t[:, 0:128], wu_t[:],
                                     start=(r == 0), stop=(r == 37))
                # DMA issue order: wv/x interleaved (phase 1b streams over
                # arriving x chunks), then the first 3 w tiles for phase 1a.
                wv_t = wvp.tile([128, NDC, 2, 512], BF16, tag="wv",
                                name="wv_t")
                x_tiles = []
                for dc in range(NDC):
                    nc.sync.dma_start(wv_t[:, dc, 0], wv[:, 0, dc])
                    x_t = xp.tile([128, S], BF16, tag=f"x{dc}",
                                  name=f"x{dc}")
                    nc.sync.dma_start(x_t[:], xt[:, dc])
                    x_tiles.append(x_t)
                for dc in range(NDC):
                    nc.sync.dma_start(wv_t[:, dc, 1], wv[:, 1, dc])

                w_tiles = {}

                def issue_w(ec):
                    w_t = wp.tile([128, NDC, 128], BF16, tag="w", name="w_t")
                    nc.sync.dma_start(w_t[:], wqk[:, ec])
                    w_tiles[ec] = w_t
                for ec in range(3):
                    issue_w(ec)

                def v_evac(ps, sc, evc):
                    for hl in range(4):
                        h = 4 * evc + hl
                        col = h * NKC + sc
                        nc.vector.tensor_scalar(
                            out=v_tiles[sc][:, 512 * evc + 128 * hl:
                                            512 * evc + 128 * (hl + 1)],
                            in0=ps[:, 128 * hl:128 * (hl + 1)],
                            scalar1=ebf_t[:, col:col + 1],
                            scalar2=None,
                            op0=MULT)

                # phase 1b (v): wave 0 = 8 chains (sc 0..7, evc=0 only:
                # the evc=1 wv halves land after the x stream), dc-outer so
                # each arriving x chunk unlocks the next step of all chains.
                ps_w = [pp.tile([128, 512], F32, tag="p", name=f"pw{i}")
                        for i in range(8)]
                for dc in range(NDC):
                    for sc in range(8):
                        nc.tensor.matmul(
                            ps_w[sc][:],
                            x_tiles[dc][:, 128 * sc:128 * (sc + 1)],
                            wv_t[:, dc, 0],
                            start=(dc == 0), stop=(dc == NDC - 1))
                for sc in range(8):
                    v_evac(ps_w[sc], sc, 0)
                # remaining v chains at full speed
                for sc, evc in ([(sc, 0) for sc in range(8, 16)]
                                + [(sc, 1) for sc in range(16)]):
                    ps = pp.tile([128, 512], F32, tag="p", name="pv")
                    for dc in range(NDC):
                        nc.tensor.matmul(
                            ps[:],
                            x_tiles[dc][:, 128 * sc:128 * (sc + 1)],
                            wv_t[:, dc, evc],
                            start=(dc == 0), stop=(dc == NDC - 1))
                    v_evac(ps, sc, evc)

                # phase 1a: qkT (16 e-chunks x 2 s-cols, accumulate 16 d)
                for ec in range(16):
                    w_t = w_tiles.pop(ec)
                    for sc4 in range(4):
                        ps = pp.tile([128, 512], F32, tag="p", name="pq")
                        for dc in range(NDC):
                            nc.tensor.matmul(
                                ps[:], w_t[:, dc],
                                x_tiles[dc][:, 512 * sc4:512 * (sc4 + 1)],
                                start=(dc == 0), stop=(dc == NDC - 1))
                        nc.scalar.copy(
                            out=qk_tiles[ec][:, 512 * sc4:512 * (sc4 + 1)],
                            in_=ps[:])
                    if ec + 3 < 16:
                        issue_w(ec + 3)

            # ---------------- phase 2 + 3: attention + out_proj ----------
            with (
                tc.tile_pool(name="attn", bufs=1) as ap,
                tc.tile_pool(name="ptp", bufs=1) as ptp,
                tc.tile_pool(name="expt", bufs=3) as ep,
                tc.tile_pool(name="rcp", bufs=2) as rp,
                tc.tile_pool(name="ost", bufs=3) as stp,
                tc.tile_pool(name="ps_sc", bufs=2, space="PSUM") as pcp,
                tc.tile_pool(name="ps_acc", bufs=2, space="PSUM") as pap,
            ):
                a_tiles = [ap.tile([128, S], BF16, tag=f"a{h}", name=f"a{h}")
                           for h in range(HPC)]
                pt_tiles = []
                for dvc in range(HPC):
                    pt_t = ptp.tile([128, D], BF16, tag=f"pt{dvc}",
                                    name=f"pt{dvc}")
                    nc.sync.dma_start(pt_t[:], ptt[:, dvc])
                    pt_tiles.append(pt_t)

                pending = deque()

                def flush(keep=0):
                    while len(pending) > keep:
                        pending.popleft()()

                def make_se_at(e_t, quad, h, at_ps, se_ps, nkc, qb):
                    def run():
                        for i, kc in enumerate(quad):
                            col = h * NKC + kc
                            hw_ = 128 if kc == 2 * qb + 1 else 256
                            o = 256 - hw_
                            st = dict(start=(kc == 0), stop=(kc == nkc - 1))
                            nc.tensor.matmul(
                                se_ps[:, o:256],
                                ebr_t[:, col:col + 1]
                                .broadcast_to([128, 128]),
                                e_t[:, 256 * i:256 * i + hw_], **st)
                            nc.tensor.matmul(
                                at_ps[:, o:256],
                                v_tiles[kc][:, 128 * h:128 * (h + 1)],
                                e_t[:, 256 * i:256 * i + hw_], **st)
                    return run

                def make_fin(h, qb, at_ps, se_ps):
                    def run():
                        rc = rp.tile([128, QW], F32, tag="rc", name="rc")
                        nc.vector.reciprocal(rc[:], se_ps[:])
                        nc.vector.tensor_mul(
                            a_tiles[h][:, QW * qb:QW * (qb + 1)],
                            at_ps[:], rc[:])
                    return run

                def op_chain(sc16, ec):
                    def run():
                        po_f = pcp.tile([128, 1024], F32, tag="sc",
                                        name="po")
                        po_t = po_f[:, 0:512]
                        for dvc in range(HPC):
                            nc.tensor.matmul(
                                po_t,
                                a_tiles[dvc][:, 128 * sc16:
                                             128 * (sc16 + 1)],
                                pt_tiles[dvc][:, 512 * ec:
                                              512 * (ec + 1)],
                                start=(dvc == 0), stop=(dvc == HPC - 1))
                        st = stp.tile([128, 512], BF16, tag="st",
                                      name="st")
                        nc.scalar.copy(out=st[:], in_=po_t)
                        nc.sync.dma_start(
                            out[128 * sc16:128 * (sc16 + 1),
                                512 * ec:512 * (ec + 1)], st[:])
                    return run

                def out_proj_chains(qb):
                    return [op_chain(2 * qb + scl, ec)
                            for scl in range(2) for ec in range(4)]

                qb_order = list(range(NQB - 1, -1, -1))
                filler = deque()
                for qi, qb in enumerate(qb_order):
                    nkc = 2 * qb + 2
                    is_last = qi == len(qb_order) - 1
                    quads = [list(range(q0, min(q0 + 4, nkc)))
                             for q0 in range(0, nkc, 4)]
                    for h in range(HPC):
                        at_ps = pap.tile([128, QW], F32, tag="at", name="at")
                        se_ps = pap.tile([128, QW], F32, tag="se", name="se")
                        for quad in quads:
                            sc_ps = pcp.tile([128, 1024], F32, tag="sc",
                                             name="sc")
                            # kc == 2qb+1 (upper diagonal chunk) only
                            # matters for queries j >= 128 of the block:
                            # process it 128 wide.
                            for i, kc in enumerate(quad):
                                half = kc == 2 * qb + 1
                                qo = QW * qb + (128 if half else 0)
                                nc.tensor.matmul(
                                    sc_ps[:, 256 * i:
                                          256 * i + (128 if half else 256)],
                                    qk_tiles[HPC + h][:, 128 * kc:
                                                      128 * (kc + 1)],
                                    qk_tiles[h][:, qo:QW * (qb + 1)],
                                    start=True, stop=True)
                            w = 256 * len(quad) - (
                                128 if 2 * qb + 1 in quad else 0)
                            e_t = ep.tile([128, 1024], BF16, tag="e",
                                          name="e")
                            nc.scalar.activation(e_t[:, :w], sc_ps[:, :w],
                                                 EXP, bias=0.0, scale=1.0)
                            for i, kc in enumerate(quad):
                                p = kc - 2 * qb
                                if p == 0:
                                    # keep j >= i
                                    nc.gpsimd.affine_select(
                                        out=e_t[:, 256 * i:256 * (i + 1)],
                                        in_=e_t[:, 256 * i:256 * (i + 1)],
                                        compare_op=GE,
                                        fill=0.0,
                                        base=0,
                                        pattern=[[1, QW]],
                                        channel_multiplier=-1)
                                elif p == 1:
                                    # half-width chunk: j' = j-128,
                                    # keep j' >= i
                                    nc.gpsimd.affine_select(
                                        out=e_t[:, 256 * i:256 * i + 128],
                                        in_=e_t[:, 256 * i:256 * i + 128],
                                        compare_op=GE,
                                        fill=0.0,
                                        base=0,
                                        pattern=[[1, 128]],
                                        channel_multiplier=-1)
                            pending.append(
                                make_se_at(e_t, quad, h, at_ps, se_ps, nkc,
                                           qb))
                            flush(keep=1)
                        # thin blocks: drip an out_proj chain of the
                        # previous block between heads to cover exp latency
                        if filler and (is_last or (nkc <= 6 and h >= 1)):
                            filler.popleft()()
                        pending.append(make_fin(h, qb, at_ps, se_ps))
                        if is_last:
                            flush()
                    while filler:
                        filler.popleft()()
                    filler.extend(out_proj_chains(qb))
                flush()
                while filler:
                    filler.popleft()()
    nc.finalize()
    return nc


def _get_nc():
    if "nc" not in _NC_CACHE:
        _NC_CACHE["nc"] = _build_nc()
    return _NC_CACHE["nc"]


def _prepare_core_inputs(x, Wqkv_w, out_proj_w, attn_bias):
    import ml_dtypes
    BF = ml_dtypes.bfloat16
    scale = 1.0 / np.sqrt(HD)
    in_maps = []
    for c in range(8):
        b, g = c // 2, c % 2
        hlo, hhi = g * EV, (g + 1) * EV
        wq = Wqkv_w[hlo:hhi] * scale            # [1024, D]
        wk = Wqkv_w[D + hlo:D + hhi]            # [1024, D]
        wvm = Wqkv_w[2 * D + hlo:2 * D + hhi]   # [1024, D]
        wqk_m = np.concatenate([wq, wk], axis=0)  # [2048, D]
        # wqk[p, ec, dc, e] = wqk_m[128*ec+e, 128*dc+p]
        wqk_t = np.ascontiguousarray(
            wqk_m.reshape(16, 128, NDC, 128).transpose(3, 0, 2, 1)
        ).astype(BF)
        # wv[p, evc, dc, c] = wvm[512*evc+c, 128*dc+p]
        wv_t = np.ascontiguousarray(
            wvm.reshape(2, 512, NDC, 128).transpose(3, 0, 2, 1)).astype(BF)
        # xt[p, dc, s] = x[b, s, 128*dc+p]
        xt = np.ascontiguousarray(
            x[b].reshape(S, NDC, 128).transpose(2, 1, 0)).astype(BF)
        # ptt[p, dvc, e] = out_proj_w[e, hlo + 128*dvc + p]
        pt = out_proj_w[:, hlo:hhi].T            # [1024, D]
        ptt = np.ascontiguousarray(
            pt.reshape(HPC, 128, D).transpose(1, 0, 2)).astype(BF)
        # ebias[i, h*16+kc] = exp(attn_bias[0, g*8+h, 0, kc*128+i])
        bias_g = attn_bias[0, g * HPC:(g + 1) * HPC, 0, :]     # [8, S]
        ebias = np.exp(np.ascontiguousarray(
            bias_g.reshape(HPC, NKC, 128).transpose(2, 0, 1)
            .reshape(128, HPC * NKC)).astype(np.float64)).astype(np.float32)
        in_maps.append({
            "xt": xt, "wqk": wqk_t, "wv": wv_t, "ptt": ptt,
            "ebias_r": ebias.astype(BF), "ebias_f": ebias,
        })
    return in_maps


def kernel(x, Wqkv_w, out_proj_w, attn_bias, key_padding_mask=None):
    """Full inputs in, full [B, S, D] float32 output out.

    key_padding_mask is all-True for this problem spec and is ignored.
    """
    global LAST_EXEC_NS, LAST_PER_CORE_NS
    from concourse.bass_utils import run_bass_kernel_spmd

    x = np.asarray(x, dtype=np.float32)
    Wqkv_w = np.asarray(Wqkv_w, dtype=np.float32)
    out_proj_w = np.asarray(out_proj_w, dtype=np.float32)
    attn_bias = np.asarray(attn_bias, dtype=np.float32)

    trace = bool(int(os.environ.get("KERNEL_TRACE", "0")))
    if trace:
        _install_ntff_hook()

    nc = _get_nc()
    in_maps = _prepare_core_inputs(x, Wqkv_w, out_proj_w, attn_bias)
    kwargs = {}
    if trace:
        kwargs.update(trace=True, trace_cores=list(range(8)))
    res = run_bass_kernel_spmd(nc, in_maps, core_ids=list(range(8)), **kwargs)
    LAST_EXEC_NS = res.exec_time_ns
    LAST_PER_CORE_NS = res.mean_exec_time_ns

    out = np.empty((B, S, D), dtype=np.float32)
    for b in range(B):
        out[b] = (res.results[2 * b]["o"].astype(np.float32)
                  + res.results[2 * b + 1]["o"].astype(np.float32))
    return out


# revision 31
# speedup vs baseline: 1.0012x; 1.0012x over previous
"""Multi-head causal attention (B=4, S=2048, D=2048, H=16) on 8 trn2 cores.

Sharding: core c handles batch b = c//2 and head-group g = c%2 (8 heads).
Each core computes q/k/v projections for its heads, causal attention, and a
partial out_proj over its dv-slice. Host sums the two partials per batch.

All matmuls in bfloat16 (same 1 cyc/row as f32r but half the bytes),
everything SBUF-resident (no DRAM round trip for q/k/v), input DMAs ordered
so the first projection chains consume x chunks as they stream in (single
in-order SP HWDGE queue), 256-wide query blocks with the upper diagonal
key chunk processed 128 wide (it is dead for the block's first 128
queries), out_proj emitted as per-head filler chains that cover softmax-exp
latency, and dep-free warm-up matmuls at t=0 that flip the PE HAM
clock-gate to 8/8 during the DMA queue spin-up window.

Device pipeline (per core):
  phase 1b: v[s, ev] = xT-chunks.T @ WvT, scaled by exp(alibi_bias[h, k])
            during PSUM evacuation (folds ALiBi into softmax via
            exp(s + b) = exp(s) * exp(b)). First 8 chains are emitted
            dc-outer so they consume x chunks as the DMA stream lands.
  phase 1a: qkT[e, s] = WqkT-chunks.T @ xT   (e: 8 q-heads then 8 k-heads)
  phase 2 per (256-query block, head): scoresT[k, q] = kT-chunk.T @ qT,
     four 128-key chunks into one [128, 1024] PSUM tile
     -> one wide ACT exp -> GPSIMD affine_select zeroes the causal
        staircase on the two diagonal chunks
     -> sumexp[*, q] += ebias-col-broadcast.T @ expT
     -> attnT[dv, q] += v'-chunk.T @ expT
     -> attnT *= 1/sumexp  (DVE reciprocal + mul)
  phase 3 (interleaved, one query block behind): O[s, e] partial
     = attnT-chunks.T @ out_projT over this core's dv-slice.
"""
import os
import sys
import types
from collections import deque

if "/opt/trn_rl_repo" not in sys.path:
    sys.path.insert(0, "/opt/trn_rl_repo")

import numpy as np

B, S, D, H = 4, 2048, 2048, 16
HD = D // H          # 128 head dim
HPC = H // 2         # 8 heads per core
EV = HPC * HD        # 1024 dv-slice per core
NKC = S // 128       # 16 key chunks
NDC = D // 128       # 16 contraction chunks
QW = 256             # query block width
NQB = S // QW        # 8 query blocks

_NC_CACHE = {}
LAST_EXEC_NS = None
LAST_PER_CORE_NS = None


def _install_ntff_hook():
    try:
        import antenv
        if "antenv.axon_hooks" in sys.modules:
            return
        mod = types.ModuleType("antenv.axon_hooks")
        state = {"hook": None}
        mod.set_axon_ntff_profile_hook = lambda h: state.__setitem__("hook", h)
        mod.get_axon_ntff_profile_hook = lambda: state["hook"]
        sys.modules["antenv.axon_hooks"] = mod
        antenv.axon_hooks = mod
        from trn_agent_boot.trn_boot import _ntff_profile_via_ctypes
        mod.set_axon_ntff_profile_hook(
            _ntff_profile_via_ctypes("/opt/axon/libaxon_pjrt.so"))
    except Exception:
        pass


def _build_nc():
    import concourse.bacc as bacc
    import concourse.mybir as mybir
    import concourse.tile as tile

    F32 = mybir.dt.float32
    BF16 = mybir.dt.bfloat16
    EXP = mybir.ActivationFunctionType.Exp
    MULT = mybir.AluOpType.mult
    GE = mybir.AluOpType.is_ge

    nc = bacc.Bacc()
    # xt[p, dc, s] = x[b, s, 128*dc+p]
    xt = nc.dram_tensor("xt", [128, NDC, S], BF16, kind="ExternalInput")
    # wqk[p, ec, dc, e] = Wqk_scaled[128*ec+e, 128*dc+p]
    wqk = nc.dram_tensor("wqk", [128, 16, NDC, 128], BF16,
                         kind="ExternalInput")
    # wv[p, evc, dc, c] = Wv[512*evc+c, 128*dc+p]
    wv = nc.dram_tensor("wv", [128, 2, NDC, 512], BF16, kind="ExternalInput")
    # ptt[p, dvc, e] = out_proj_w[e, 128*dvc+p]  (within this core's slice)
    ptt = nc.dram_tensor("ptt", [128, HPC, D], BF16, kind="ExternalInput")
    # ebias[i, h*16+kc] = exp(attn_bias[h, kc*128+i])
    ebias_r = nc.dram_tensor("ebias_r", [128, HPC * NKC], BF16,
                             kind="ExternalInput")
    ebias_f = nc.dram_tensor("ebias_f", [128, HPC * NKC], F32,
                             kind="ExternalInput")
    out = nc.dram_tensor("o", [S, D], BF16, kind="ExternalOutput")

    with tile.TileContext(nc) as tc:
        with (
            tc.tile_pool(name="consts", bufs=1) as cp,
            tc.tile_pool(name="qk", bufs=1) as qkp,
            tc.tile_pool(name="vv", bufs=1) as vp,
        ):
            ebr_t = cp.tile([128, HPC * NKC], BF16, tag="ebr", name="ebr")
            ebf_t = cp.tile([128, HPC * NKC], F32, tag="ebf", name="ebf")
            nc.sync.dma_start(ebr_t[:], ebias_r[:])
            nc.sync.dma_start(ebf_t[:], ebias_f[:])

            v_tiles = [vp.tile([128, EV], BF16, tag=f"v{sc}", name=f"v{sc}")
                       for sc in range(NKC)]
            qk_tiles = [qkp.tile([128, S], BF16, tag=f"qk{ec}",
                                 name=f"qk{ec}")
                        for ec in range(16)]

            # ---------------- phase 1: projections ----------------
            with (
                tc.tile_pool(name="xp", bufs=1) as xp,
                tc.tile_pool(name="wvp", bufs=1) as wvp,
                tc.tile_pool(name="wp", bufs=3) as wp,
                tc.tile_pool(name="ps1", bufs=8, space="PSUM") as pp,
            ):
                # warm-up: dep-free dummy matmuls fill the DMA queue spin-up
                # window and flip the PE HAM clock-gate to 8/8 before real
                # work arrives. Inputs are uninitialized SBUF; output unused.
                wu_t = xp.tile([128, 512], BF16, tag="wu", name="wu")
                nc.vector.memset(wu_t[:], 0.0)
                wu_ps = pp.tile([128, 512], F32, tag="p", name="wu_ps")
                for r in range(20):
                    nc.tensor.matmul(wu_ps[:], wu_t[:, 0:128], wu_t[:],
                                     start=(r == 0), stop=(r == 19))
                # DMA issue order: wv/x interleaved (phase 1b streams over
                # arriving x chunks), then the first 3 w tiles for phase 1a.
                wv_t = wvp.tile([128, NDC, 2, 512], BF16, tag="wv",
                                name="wv_t")
                x_tiles = []
                for dc in range(NDC):
                    nc.sync.dma_start(wv_t[:, dc, 0], wv[:, 0, dc])
                    x_t = xp.tile([128, S], BF16, tag=f"x{dc}",
                                  name=f"x{dc}")
                    nc.sync.dma_start(x_t[:], xt[:, dc])
                    x_tiles.append(x_t)
                for dc in range(NDC):
                    nc.sync.dma_start(wv_t[:, dc, 1], wv[:, 1, dc])

                w_tiles = {}

                def issue_w(ec):
                    w_t = wp.tile([128, NDC, 128], BF16, tag="w", name="w_t")
                    nc.sync.dma_start(w_t[:], wqk[:, ec])
                    w_tiles[ec] = w_t
                for ec in range(3):
                    issue_w(ec)

                def v_evac(ps, sc, evc):
                    for hl in range(4):
                        h = 4 * evc + hl
                        col = h * NKC + sc
                        nc.vector.tensor_scalar(
                            out=v_tiles[sc][:, 512 * evc + 128 * hl:
                                            512 * evc + 128 * (hl + 1)],
                            in0=ps[:, 128 * hl:128 * (hl + 1)],
                            scalar1=ebf_t[:, col:col + 1],
                            scalar2=None,
                            op0=MULT)

                # phase 1b (v): wave 0 = 8 chains (sc 0..7, evc=0 only:
                # the evc=1 wv halves land after the x stream), dc-outer so
                # each arriving x chunk unlocks the next step of all chains.
                ps_w = [pp.tile([128, 512], F32, tag="p", name=f"pw{i}")
                        for i in range(8)]
                for dc in range(NDC):
                    for sc in range(8):
                        nc.tensor.matmul(
                            ps_w[sc][:],
                            x_tiles[dc][:, 128 * sc:128 * (sc + 1)],
                            wv_t[:, dc, 0],
                            start=(dc == 0), stop=(dc == NDC - 1))
                for sc in range(8):
                    v_evac(ps_w[sc], sc, 0)
                # remaining v chains at full speed
                for sc, evc in ([(sc, 0) for sc in range(8, 16)]
                                + [(sc, 1) for sc in range(16)]):
                    ps = pp.tile([128, 512], F32, tag="p", name="pv")
                    for dc in range(NDC):
                        nc.tensor.matmul(
                            ps[:],
                            x_tiles[dc][:, 128 * sc:128 * (sc + 1)],
                            wv_t[:, dc, evc],
                            start=(dc == 0), stop=(dc == NDC - 1))
                    v_evac(ps, sc, evc)

                # phase 1a: qkT (16 e-chunks x 2 s-cols, accumulate 16 d)
                for ec in range(16):
                    w_t = w_tiles.pop(ec)
                    for sc4 in range(4):
                        ps = pp.tile([128, 512], F32, tag="p", name="pq")
                        for dc in range(NDC):
                            nc.tensor.matmul(
                                ps[:], w_t[:, dc],
                                x_tiles[dc][:, 512 * sc4:512 * (sc4 + 1)],
                                start=(dc == 0), stop=(dc == NDC - 1))
                        nc.scalar.copy(
                            out=qk_tiles[ec][:, 512 * sc4:512 * (sc4 + 1)],
                            in_=ps[:])
                    if ec + 3 < 16:
                        issue_w(ec + 3)

            # ---------------- phase 2 + 3: attention + out_proj ----------
            with (
                tc.tile_pool(name="attn", bufs=1) as ap,
                tc.tile_pool(name="ptp", bufs=1) as ptp,
                tc.tile_pool(name="expt", bufs=3) as ep,
                tc.tile_pool(name="rcp", bufs=2) as rp,
                tc.tile_pool(name="ost", bufs=3) as stp,
                tc.tile_pool(name="ps_sc", bufs=2, space="PSUM") as pcp,
                tc.tile_pool(name="ps_acc", bufs=2, space="PSUM") as pap,
            ):
                a_tiles = [ap.tile([128, S], BF16, tag=f"a{h}", name=f"a{h}")
                           for h in range(HPC)]
                pt_tiles = []
                for dvc in range(HPC):
                    pt_t = ptp.tile([128, D], BF16, tag=f"pt{dvc}",
                                    name=f"pt{dvc}")
                    nc.sync.dma_start(pt_t[:], ptt[:, dvc])
                    pt_tiles.append(pt_t)

                pending = deque()

                def flush(keep=0):
                    while len(pending) > keep:
                        pending.popleft()()

                def make_se_at(e_t, quad, h, at_ps, se_ps, nkc, qb):
                    def run():
                        for i, kc in enumerate(quad):
                            col = h * NKC + kc
                            hw_ = 128 if kc == 2 * qb + 1 else 256
                            o = 256 - hw_
                            st = dict(start=(kc == 0), stop=(kc == nkc - 1))
                            nc.tensor.matmul(
                                se_ps[:, o:256],
                                ebr_t[:, col:col + 1]
                                .broadcast_to([128, 128]),
                                e_t[:, 256 * i:256 * i + hw_], **st)
                            nc.tensor.matmul(
                                at_ps[:, o:256],
                                v_tiles[kc][:, 128 * h:128 * (h + 1)],
                                e_t[:, 256 * i:256 * i + hw_], **st)
                    return run

                def make_fin(h, qb, at_ps, se_ps):
                    def run():
                        rc = rp.tile([128, QW], F32, tag="rc", name="rc")
                        nc.vector.reciprocal(rc[:], se_ps[:])
                        nc.vector.tensor_mul(
                            a_tiles[h][:, QW * qb:QW * (qb + 1)],
                            at_ps[:], rc[:])
                    return run

                def op_chain(sc16, ec):
                    def run():
                        po_f = pcp.tile([128, 1024], F32, tag="sc",
                                        name="po")
                        po_t = po_f[:, 0:512]
                        for dvc in range(HPC):
                            nc.tensor.matmul(
                                po_t,
                                a_tiles[dvc][:, 128 * sc16:
                                             128 * (sc16 + 1)],
                                pt_tiles[dvc][:, 512 * ec:
                                              512 * (ec + 1)],
                                start=(dvc == 0), stop=(dvc == HPC - 1))
                        st = stp.tile([128, 512], BF16, tag="st",
                                      name="st")
                        nc.scalar.copy(out=st[:], in_=po_t)
                        nc.sync.dma_start(
                            out[128 * sc16:128 * (sc16 + 1),
                                512 * ec:512 * (ec + 1)], st[:])
                    return run

                def out_proj_chains(qb):
                    return [op_chain(2 * qb + scl, ec)
                            for scl in range(2) for ec in range(4)]

                qb_order = list(range(NQB - 1, -1, -1))
                filler = deque()
                for qi, qb in enumerate(qb_order):
                    nkc = 2 * qb + 2
                    is_last = qi == len(qb_order) - 1
                    quads = [list(range(q0, min(q0 + 4, nkc)))
                             for q0 in range(0, nkc, 4)]
                    for h in range(HPC):
                        at_ps = pap.tile([128, QW], F32, tag="at", name="at")
                        se_ps = pap.tile([128, QW], F32, tag="se", name="se")
                        for quad in quads:
                            sc_ps = pcp.tile([128, 1024], F32, tag="sc",
                                             name="sc")
                            # kc == 2qb+1 (upper diagonal chunk) only
                            # matters for queries j >= 128 of the block:
                            # process it 128 wide.
                            for i, kc in enumerate(quad):
                                half = kc == 2 * qb + 1
                                qo = QW * qb + (128 if half else 0)
                                nc.tensor.matmul(
                                    sc_ps[:, 256 * i:
                                          256 * i + (128 if half else 256)],
                                    qk_tiles[HPC + h][:, 128 * kc:
                                                      128 * (kc + 1)],
                                    qk_tiles[h][:, qo:QW * (qb + 1)],
                                    start=True, stop=True)
                            w = 256 * len(quad) - (
                                128 if 2 * qb + 1 in quad else 0)
                            e_t = ep.tile([128, 1024], BF16, tag="e",
                                          name="e")
                            nc.scalar.activation(e_t[:, :w], sc_ps[:, :w],
                                                 EXP, bias=0.0, scale=1.0)
                            for i, kc in enumerate(quad):
                                p = kc - 2 * qb
                                if p == 0:
                                    # keep j >= i
                                    nc.gpsimd.affine_select(
                                        out=e_t[:, 256 * i:256 * (i + 1)],
                                        in_=e_t[:, 256 * i:256 * (i + 1)],
                                        compare_op=GE,
                                        fill=0.0,
                                        base=0,
                                        pattern=[[1, QW]],
                                        channel_multiplier=-1)
                                elif p == 1:
                                    # half-width chunk: j' = j-128,
                                    # keep j' >= i
                                    nc.gpsimd.affine_select(
                                        out=e_t[:, 256 * i:256 * i + 128],
                                        in_=e_t[:, 256 * i:256 * i + 128],
                                        compare_op=GE,
                                        fill=0.0,
                                        base=0,
                                        pattern=[[1, 128]],
                                        channel_multiplier=-1)
                            pending.append(
                                make_se_at(e_t, quad, h, at_ps, se_ps, nkc,
                                           qb))
                            flush(keep=1)
                        # thin blocks: drip an out_proj chain of the
                        # previous block between heads to cover exp latency
                        if filler and (is_last or (nkc <= 6 and h >= 1)):
                            filler.popleft()()
                        pending.append(make_fin(h, qb, at_ps, se_ps))
                        if is_last:
                            flush()
                    while filler:
                        filler.popleft()()
                    filler.extend(out_proj_chains(qb))
                flush()
                while filler:
                    filler.popleft()()
    nc.finalize()
    return nc


def _get_nc():
    if "nc" not in _NC_CACHE:
        _NC_CACHE["nc"] = _build_nc()
    return _NC_CACHE["nc"]


def _prepare_core_inputs(x, Wqkv_w, out_proj_w, attn_bias):
    import ml_dtypes
    BF = ml_dtypes.bfloat16
    scale = 1.0 / np.sqrt(HD)
    in_maps = []
    for c in range(8):
        b, g = c // 2, c % 2
        hlo, hhi = g * EV, (g + 1) * EV
        wq = Wqkv_w[hlo:hhi] * scale            # [1024, D]
        wk = Wqkv_w[D + hlo:D + hhi]            # [1024, D]
        wvm = Wqkv_w[2 * D + hlo:2 * D + hhi]   # [1024, D]
        wqk_m = np.concatenate([wq, wk], axis=0)  # [2048, D]
        # wqk[p, ec, dc, e] = wqk_m[128*ec+e, 128*dc+p]
        wqk_t = np.ascontiguousarray(
            wqk_m.reshape(16, 128, NDC, 128).transpose(3, 0, 2, 1)
        ).astype(BF)
        # wv[p, evc, dc, c] = wvm[512*evc+c, 128*dc+p]
        wv_t = np.ascontiguousarray(
            wvm.reshape(2, 512, NDC, 128).transpose(3, 0, 2, 1)).astype(BF)
        # xt[p, dc, s] = x[b, s, 128*dc+p]
        xt = np.ascontiguousarray(
            x[b].reshape(S, NDC, 128).transpose(2, 1, 0)).astype(BF)
        # ptt[p, dvc, e] = out_proj_w[e, hlo + 128*dvc + p]
        pt = out_proj_w[:, hlo:hhi].T            # [1024, D]
        ptt = np.ascontiguousarray(
            pt.reshape(HPC, 128, D).transpose(1, 0, 2)).astype(BF)
        # ebias[i, h*16+kc] = exp(attn_bias[0, g*8+h, 0, kc*128+i])
        bias_g = attn_bias[0, g * HPC:(g + 1) * HPC, 0, :]     # [8, S]
        ebias = np.exp(np.ascontiguousarray(
            bias_g.reshape(HPC, NKC, 128).transpose(2, 0, 1)
            .reshape(128, HPC * NKC)).astype(np.float64)).astype(np.float32)
        in_maps.append({
            "xt": xt, "wqk": wqk_t, "wv": wv_t, "ptt": ptt,
            "ebias_r": ebias.astype(BF), "ebias_f": ebias,
        })
    return in_maps


def kernel(x, Wqkv_w, out_proj_w, attn_bias, key_padding_mask=None):
    """Full inputs in, full [B, S, D] float32 output out.

    key_padding_mask is all-True for this problem spec and is ignored.
    """
    global LAST_EXEC_NS, LAST_PER_CORE_NS
    from concourse.bass_utils import run_bass_kernel_spmd

    x = np.asarray(x, dtype=np.float32)
    Wqkv_w = np.asarray(Wqkv_w, dtype=np.float32)
    out_proj_w = np.asarray(out_proj_w, dtype=np.float32)
    attn_bias = np.asarray(attn_bias, dtype=np.float32)

    trace = bool(int(os.environ.get("KERNEL_TRACE", "0")))
    if trace:
        _install_ntff_hook()

    nc = _get_nc()
    in_maps = _prepare_core_inputs(x, Wqkv_w, out_proj_w, attn_bias)
    kwargs = {}
    if trace:
        kwargs.update(trace=True, trace_cores=list(range(8)))
    res = run_bass_kernel_spmd(nc, in_maps, core_ids=list(range(8)), **kwargs)
    LAST_EXEC_NS = res.exec_time_ns
    LAST_PER_CORE_NS = res.mean_exec_time_ns

    out = np.empty((B, S, D), dtype=np.float32)
    for b in range(B):
        out[b] = (res.results[2 * b]["o"].astype(np.float32)
                  + res.results[2 * b + 1]["o"].astype(np.float32))
    return out
